# revision 11
# baseline (speedup 1.0000x reference)
"""Trainium2 Bass kernel for nn_BartCrossAttention (B=4, L=1024, D=1024, H=16, HD=64).

v3 sharding: core c -> (batch b = c//2, head-half j = c%2). Each core computes
heads [8j, 8j+8) for ALL 1024 query tokens of its batch, including the K/V/Q
projections restricted to its 512 features, then a PARTIAL out-projection
(contracting only its 512 ctx features). The host sums the two partial outputs
per batch and adds out_bias. No collectives.

v3 changes vs v2 (203.5us):
  - host pre-transposes hid/kv -> hidT/kvT shipped d-major: kills all 128
    PE transposes + identity build, shortens the prologue critical path
  - wk/wq shipped in per-f-block layout [p, f, dd, c] so the f=0 slices can
    land first and K0/Q0 projections start as soon as kvT/hidT stream in
  - DMA issue split across sync/gpsimd/scalar queues, critical tensors first
  - K0/K1/Q0 in the prologue; all other projections spread over the head
    loop as full-array filler (keeps PE HAM at 8/8), one v-site per h0 slot
    (matches the AV consumption deadline exactly), 2 K/Q sites per head after
  - normalization: reciprocal straight off the psum denominator row, then a
    PE broadcast (ones[1,64] x recip row, f32r) + one DVE evict-multiply per
    half; c0 first so the next head's AV frees a ctx tile a slot earlier
  - out partials DMA'd as full [128,1024] tiles round-robin on 3 queues
"""
import sys

for _p in ("/opt/trn_rl_repo",):
    if _p not in sys.path:
        sys.path.insert(0, _p)

import numpy as np
import ml_dtypes

import concourse.bass as bass
import concourse.mybir as mybir
import concourse.tile as tile
from concourse import bacc
import concourse.bass_utils as bass_utils

F32 = mybir.dt.float32
F32R = mybir.dt.float32r
BF16 = mybir.dt.bfloat16
NPBF16 = ml_dtypes.bfloat16

P = 128
D = 1024        # model dim
H = 16          # heads (global)
HPC = 8         # heads per core
FPC = 512       # features per core
NCORES = 8
B, LQ, LK = 4, 1024, 1024

_CACHE = {}


def _build_core_program():
    nc = bacc.Bacc("TRN2", target_bir_lowering=False, debug=False,
                   num_devices=NCORES)

    kvT_s = nc.dram_tensor("kvT_s", [D, LK], BF16, kind="ExternalInput")
    hidT_s = nc.dram_tensor("hidT_s", [D, LQ], BF16, kind="ExternalInput")
    wv_t = nc.dram_tensor("wv_t", [D, FPC], BF16, kind="ExternalInput")
    wkf_t = nc.dram_tensor("wkf_t", [P, 4, 8, P], BF16, kind="ExternalInput")
    wqf_t = nc.dram_tensor("wqf_t", [P, 4, 8, P], BF16, kind="ExternalInput")
    wo_t = nc.dram_tensor("wo_t", [FPC, D], BF16, kind="ExternalInput")
    qkb_d = nc.dram_tensor("qkb", [P, 8], F32, kind="ExternalInput")
    vb_d = nc.dram_tensor("vb", [1, FPC], F32, kind="ExternalInput")
    out_s = nc.dram_tensor("out_s", [LQ, D], BF16, kind="ExternalOutput")

    Exp = mybir.ActivationFunctionType.Exp
    Ident = mybir.ActivationFunctionType.Identity
    add = mybir.AluOpType.add
    mult = mybir.AluOpType.mult

    with tile.TileContext(nc) as tc:
        with (
            tc.tile_pool(name="setup", bufs=1) as setup,
            tc.tile_pool(name="big", bufs=1) as big,
        ):
            # ---- persistent tiles ----
            kvT = big.tile([P, 8, LK], BF16, tag="kvT")    # kv^T [1024,1024]
            hidT = big.tile([P, 8, LQ], BF16, tag="hidT")  # hid^T
            wv = setup.tile([P, 8, FPC], BF16, tag="wv")
            wk = setup.tile([P, 4, 8, P], BF16, tag="wk")  # [p, f, dd, c]
            wq = setup.tile([P, 4, 8, P], BF16, tag="wq")
            wo = setup.tile([P, 4, D], BF16, tag="wo")
            KT = big.tile([P, 4, LK], BF16, tag="KT")      # K^T [512,1024]
            qT = big.tile([P, 4, LQ], BF16, tag="qT")      # Q^T [512,1024]
            v65 = big.tile([P, 8, HPC * 65], BF16, tag="v65")
            ctxT = big.tile([P, 4, LQ], BF16, tag="ctxT")  # ctx^T [512,1024]
            qkb_sb = setup.tile([P, 8], F32, tag="qkb")
            qb_sb = qkb_sb[:, 0:4]
            kb_sb = qkb_sb[:, 4:8]
            vb_row = setup.tile([1, FPC], F32, tag="vb_row")

            # ---- all input DMAs up front, split across 3 issue engines so
            # the critical tensors (wkf0/kvT then wqf0/hidT) stream first ----
            kvT_r = kvT_s.ap().rearrange("(dd p) t -> p dd t", p=P)
            hidT_r = hidT_s.ap().rearrange("(dd p) t -> p dd t", p=P)
            wkf_r = wkf_t.ap()
            wqf_r = wqf_t.ap()

            # per-dd-slab DMAs so the dd-major prologue projections can
            # start accumulating as each slab lands; slabs 0-3 on sync,
            # 4-7 on gpsimd so the two queues stream in parallel
            wv_r = wv_t.ap().rearrange("(dd p) o -> p dd o", p=P)
            nc.sync.dma_start(qkb_sb[:], qkb_d.ap())
            nc.sync.dma_start(wk[:, 0:1, :, :], wkf_r[:, 0:1, :, :])
            for dd in range(4):
                nc.sync.dma_start(kvT[:, dd:dd + 1, :],
                                  kvT_r[:, dd:dd + 1, :])
                nc.sync.dma_start(hidT[:, dd:dd + 1, :],
                                  hidT_r[:, dd:dd + 1, :])
            nc.sync.dma_start(wk[:, 1:2, :, :], wkf_r[:, 1:2, :, :])
            nc.sync.dma_start(wv[:, 0:4, :], wv_r[:, 0:4, :])
            nc.sync.dma_start(wk[:, 2:4, :, :], wkf_r[:, 2:4, :, :])

            nc.gpsimd.dma_start(wq[:, 0:1, :, :], wqf_r[:, 0:1, :, :])
            for dd in range(4, 8):
                nc.gpsimd.dma_start(kvT[:, dd:dd + 1, :],
                                    kvT_r[:, dd:dd + 1, :])
                nc.gpsimd.dma_start(hidT[:, dd:dd + 1, :],
                                    hidT_r[:, dd:dd + 1, :])
            nc.gpsimd.dma_start(wv[:, 4:8, :], wv_r[:, 4:8, :])
            nc.gpsimd.dma_start(vb_row[:], vb_d.ap())
            nc.gpsimd.dma_start(wq[:, 1:4, :, :], wqf_r[:, 1:4, :, :])
            nc.gpsimd.dma_start(wo[:], wo_t.ap().rearrange(
                "(dd p) o -> p dd o", p=P))

            # ---- small setup (gpsimd after its DMA issues; all early) ----
            vbB = setup.tile([P, FPC], F32, tag="vbB")
            nc.gpsimd.partition_broadcast(vbB[:], vb_row[:])

            # ones column (col 64 of each head block) for denominators
            onesF = setup.tile([P, 64], F32, tag="onesF")
            nc.gpsimd.memset(onesF[:], 1.0)
            nc.vector.tensor_copy(
                v65[:].rearrange("p t (h x) -> p t h x", x=65)[:, :, :, 64:65],
                onesF[:].rearrange("p (t h x) -> p t h x", t=8, h=8))

            # ---- projection emitters ----
            def emit_vproj(ti, pool):
                pv = pool.tile([P, FPC], F32, tag="pp", bufs=None,
                               name=f"pv{ti}")
                for dd in range(8):
                    nc.tensor.matmul(
                        pv[:],
                        kvT[:, dd, ti * P:(ti + 1) * P],
                        wv[:, dd, :],
                        start=(dd == 0), stop=(dd == 7),
                    )
                dst = v65[:].rearrange("p t (h x) -> p t h x", x=65)[
                    :, ti, :, 0:64]
                nc.vector.tensor_tensor(dst, pv[:], vbB[:], add)

            def emit_kproj(f, ck, pool):
                pk = pool.tile([P, FPC], F32, tag="pp", bufs=None,
                               name=f"pk{f}_{ck}")
                for dd in range(8):
                    nc.tensor.matmul(
                        pk[:],
                        wk[:, f, dd, :],
                        kvT[:, dd, ck * 512:(ck + 1) * 512],
                        start=(dd == 0), stop=(dd == 7),
                    )
                nc.vector.tensor_scalar(
                    KT[:, f, ck * 512:(ck + 1) * 512], pk[:],
                    kb_sb[:, f:f + 1], None, add)

            def emit_qproj(f, ck, pool):
                pq = pool.tile([P, FPC], F32, tag="pp", bufs=None,
                               name=f"pq{f}_{ck}")
                for dd in range(8):
                    nc.tensor.matmul(
                        pq[:],
                        wq[:, f, dd, :],
                        hidT[:, dd, ck * 512:(ck + 1) * 512],
                        start=(dd == 0), stop=(dd == 7),
                    )
                nc.vector.tensor_scalar(
                    qT[:, f, ck * 512:(ck + 1) * 512], pq[:],
                    qb_sb[:, f:f + 1], None, add)

            # ---- prologue: K f0/f1 + Q f0, dd-major so each matmul only
            # needs one kvT/hidT slab (starts as soon as slabs land) ----
            DD = [0, 4, 1, 5, 2, 6, 3, 7]  # matches 2-queue arrival order
            with tc.tile_pool(name="psmmA", bufs=6, space="PSUM") as psmmA:
                ksites = [(0, 0), (0, 1)]
                qsites = [(0, 0), (0, 1)]
                ktiles = {s: psmmA.tile([P, FPC], F32, tag="pp",
                                        name=f"pk{s[0]}_{s[1]}")
                          for s in ksites}
                qtiles = {s: psmmA.tile([P, FPC], F32, tag="pp",
                                        name=f"pq{s[0]}_{s[1]}")
                          for s in qsites}
                for i, dd in enumerate(DD):
                    for (f, ck) in ksites:
                        nc.tensor.matmul(
                            ktiles[(f, ck)][:],
                            wk[:, f, dd, :],
                            kvT[:, dd, ck * 512:(ck + 1) * 512],
                            start=(i == 0), stop=(i == 7),
                        )
                    for (f, ck) in qsites:
                        nc.tensor.matmul(
                            qtiles[(f, ck)][:],
                            wq[:, f, dd, :],
                            hidT[:, dd, ck * 512:(ck + 1) * 512],
                            start=(i == 0), stop=(i == 7),
                        )
                for (f, ck) in ksites:
                    nc.vector.tensor_scalar(
                        KT[:, f, ck * 512:(ck + 1) * 512],
                        ktiles[(f, ck)][:], kb_sb[:, f:f + 1], None, add)
                for (f, ck) in qsites:
                    nc.vector.tensor_scalar(
                        qT[:, f, ck * 512:(ck + 1) * 512],
                        qtiles[(f, ck)][:], qb_sb[:, f:f + 1], None, add)

            # filler schedule: (h, t) -> emitter. h0 carries one v-site per
            # slot (AV(h0,t) consumes v65[t-1], so site t feeds slot t+1);
            # later heads carry the remaining K/Q f-sites, 2 per head.
            # all 8 v-sites must complete inside h0 (its tail AVs read
            # v65[:,6]/[:,7]); wv lands ~26us so nothing before slot (0,2)
            FILLERS = {
                (0, 0): [("k", 1, 0)],
                (0, 1): [("k", 1, 1)],
                (0, 2): [("v", 0, None)],
                (0, 3): [("v", 1, None), ("v", 2, None)],
                (0, 4): [("v", 3, None)],
                (0, 5): [("v", 4, None), ("v", 5, None)],
                (0, 6): [("v", 6, None)],
                (0, 7): [("v", 7, None)],
                (1, 2): [("q", 1, 0)],
                (1, 5): [("q", 1, 1)],
                (2, 2): [("k", 2, 0)],
                (2, 5): [("k", 2, 1)],
                (3, 2): [("q", 2, 0)],
                (3, 5): [("q", 2, 1)],
                (4, 2): [("k", 3, 0)],
                (4, 5): [("k", 3, 1)],
                (5, 2): [("q", 3, 0)],
                (5, 5): [("q", 3, 1)],
            }

            def emit_filler(h, t, pool):
                for kind, a, b in FILLERS.get((h, t), ()):
                    if kind == "v":
                        emit_vproj(a, pool)
                    elif kind == "k":
                        emit_kproj(a, b, pool)
                    else:
                        emit_qproj(a, b, pool)

            # ---- attention main loop ----
            with (
                tc.tile_pool(name="scp", bufs=2, space="PSUM") as scp,
                tc.tile_pool(name="ctxp", bufs=3, space="PSUM") as ctxp,
                tc.tile_pool(name="psmmB", bufs=1, space="PSUM") as psmmB,
                tc.tile_pool(name="lp", bufs=1) as lp,
            ):
                def emit_norm_recips(h, ctx_pair):
                    # denominator row: psum -> sbuf stage, then fast recip
                    # (custom-DVE recip can't read PSUM directly)
                    rcps = []
                    for c in range(2):
                        stg = lp.tile([1, FPC], F32, tag=f"stg{c}",
                                      name=f"stg{h}_{c}")
                        nc.vector.tensor_copy(stg[:], ctx_pair[c][64:65, :])
                        rcp = lp.tile([1, FPC], F32, tag=f"rcp{c}",
                                      name=f"rcp{h}_{c}")
                        nc.vector.reciprocal_approx_fast(rcp[:], stg[:])
                        rcps.append(rcp)
                    return rcps

                def emit_norm_half(h, ctx_pair, rcps, c):
                    # broadcast the recip row to 64 partitions (gpsimd),
                    # then one DVE evict-multiply into ctxT (bf16)
                    f, rb = h // 2, 64 * (h % 2)
                    bcb = lp.tile([64, FPC], F32, tag=f"bcb{c}",
                                    bufs=2, name=f"bcb{h}_{c}")
                    nc.gpsimd.partition_broadcast(bcb[:], rcps[c][:])
                    nc.vector.tensor_tensor(
                        ctxT[rb:rb + 64, f, c * 512:(c + 1) * 512],
                        ctx_pair[c][0:64, :], bcb[:], mult)

                prev = None  # (h, ctx_pair, rcps) pending normalization
                for h in range(HPC):
                    f, rb = h // 2, 64 * (h % 2)
                    ctx_pair = [ctxp.tile([65, FPC], F32, tag="ctx",
                                          name=f"ctx{h}_{c}")
                                for c in range(2)]
                    ats = []
                    for t in range(8):
                        sc = scp.tile([P, 2 * FPC], F32, tag="sc",
                                      name=f"sc{h}_{t}")
                        for c in range(2):
                            nc.tensor.matmul(
                                sc[:, c * FPC:(c + 1) * FPC],
                                KT[rb:rb + 64, f, t * P:(t + 1) * P],
                                qT[rb:rb + 64, f, c * 512:(c + 1) * 512],
                                start=True, stop=True,
                            )
                        # one 1024-wide exp per t (full ACT column rate)
                        at = lp.tile([P, 2 * FPC], BF16, tag="at",
                                     bufs=4, name=f"at{h}_{t}")
                        nc.scalar.activation(at[:], sc[:], Exp)
                        ats.append(at)
                        # pipelined normalization of the previous head:
                        # c0 at t==0 (frees a ctx tile early), c1 at t==1
                        if prev is not None and t <= 1:
                            ph, pctx, prcps = prev
                            emit_norm_half(ph, pctx, prcps, t)
                            if t == 1:
                                prev = None
                        # full-array projection filler keeps the PE HAM
                        # activity monitor warm (2.4 GHz) during the
                        # half-array attention matmuls
                        emit_filler(h, t, psmmB)
                        # AV lags TWO t-slots: the exp it consumes finished
                        # a full slot ago, so the PE never waits on ACT
                        if t >= 2:
                            for c in range(2):
                                nc.tensor.matmul(
                                    ctx_pair[c][:],
                                    v65[:, t - 2, h * 65:(h + 1) * 65],
                                    ats[t - 2][:, c * FPC:(c + 1) * FPC],
                                    start=(t == 2), stop=False,
                                )
                    for tt in (6, 7):
                        for c in range(2):
                            nc.tensor.matmul(
                                ctx_pair[c][:],
                                v65[:, tt, h * 65:(h + 1) * 65],
                                ats[tt][:, c * FPC:(c + 1) * FPC],
                                start=False, stop=(tt == 7),
                            )
                    rcps = emit_norm_recips(h, ctx_pair)
                    prev = (h, ctx_pair, rcps)
                # final head's normalization (fast path, both halves)
                ph, pctx, prcps = prev
                emit_norm_half(ph, pctx, prcps, 0)
                emit_norm_half(ph, pctx, prcps, 1)

            # ---- epilogue: partial out projection (no bias; host adds) ----
            out_r = out_s.ap().rearrange("(mm p) d -> p mm d", p=P)
            # out-proj in (m, half) groups: the fj=0..2 partial sums only
            # need ctxT f0-f2 (ready before the last head's norm), so 8
            # groups' worth of independent matmuls hide the final norm
            # latency; fj=3 + eviction + DMA follow per group
            with (
                tc.tile_pool(name="pop", bufs=8, space="PSUM") as pop,
                tc.tile_pool(name="outp", bufs=3) as outp,
            ):
                groups = [(m, half) for m in range(8) for half in range(2)]
                po_t = {}
                ot_t = {}

                def emit_partial(m, half):
                    po = pop.tile([P, FPC], F32, tag="po",
                                  name=f"po{m}_{half}")
                    po_t[(m, half)] = po
                    for fj in range(3):
                        nc.tensor.matmul(
                            po[:],
                            ctxT[:, fj, m * P:(m + 1) * P],
                            wo[:, fj, half * 512:(half + 1) * 512],
                            start=(fj == 0), stop=False,
                        )

                for (m, half) in groups[:8]:
                    emit_partial(m, half)
                for i, (m, half) in enumerate(groups):
                    po = po_t[(m, half)]
                    nc.tensor.matmul(
                        po[:],
                        ctxT[:, 3, m * P:(m + 1) * P],
                        wo[:, 3, half * 512:(half + 1) * 512],
                        start=False, stop=True,
                    )
                    if half == 0:
                        ot_t[m] = outp.tile([P, D], BF16, tag="ot",
                                            name=f"ot{m}")
                        nc.scalar.activation(
                            ot_t[m][:, 0:512], po[:], Ident)
                    else:
                        nc.vector.tensor_copy(
                            ot_t[m][:, 512:1024], po[:])
                    nc.sync.dma_start(
                        out_r[:, m, half * 512:(half + 1) * 512],
                        ot_t[m][:, half * 512:(half + 1) * 512])
                    if i + 8 < 16:
                        emit_partial(*groups[i + 8])

    nc.compile()
    return nc


def _prep_inputs(hidden_states, key_value_states, q_weight, q_bias,
                 kv_weight, kv_bias, out_weight, out_bias):
    f32 = np.float32
    hid = np.asarray(hidden_states, f32).reshape(B, LQ, D)
    kv = np.asarray(key_value_states, f32).reshape(B, LK, D)
    scale = f32(1.0 / 8.0)

    # de-interleave kv rows: row e <-> (h=e//128, j=(e%128)//64, d=e%64)
    e = np.arange(2 * D)
    kmask = (e % 128) < 64
    kidx, vidx = e[kmask], e[~kmask]
    kvw = np.asarray(kv_weight, f32)
    kvb = np.asarray(kv_bias, f32)

    wq_full = (np.asarray(q_weight, f32) * scale).T      # [D, D] d x feat
    wk_full = kvw[kidx].T                                # [D, D]
    wv_full = kvw[vidx].T
    wo_full = np.asarray(out_weight, f32).T              # [D, D] feat x out
    qb_full = np.asarray(q_bias, f32) * scale
    kb_full = kvb[kidx]
    vb_full = kvb[vidx]

    def fblock(w):
        # [D, FPC] (d, feat) -> [128 p, 4 f, 8 dd, 128 c] bf16
        return np.ascontiguousarray(
            w.reshape(8, P, 4, P).transpose(1, 2, 0, 3).astype(NPBF16))

    jmaps = []
    for j in range(2):
        s = slice(j * FPC, (j + 1) * FPC)
        jmaps.append({
            "wqf_t": fblock(wq_full[:, s]),
            "wkf_t": fblock(wk_full[:, s]),
            "wv_t": np.ascontiguousarray(wv_full[:, s].astype(NPBF16)),
            "wo_t": np.ascontiguousarray(wo_full[s, :].astype(NPBF16)),
            "qkb": np.ascontiguousarray(np.concatenate(
                [qb_full[s].reshape(4, P).T, kb_full[s].reshape(4, P).T],
                axis=1)),
            "vb": np.ascontiguousarray(vb_full[s].reshape(1, FPC)),
        })
    hidT = [np.ascontiguousarray(hid[b].T.astype(NPBF16)) for b in range(B)]
    kvT = [np.ascontiguousarray(kv[b].T.astype(NPBF16)) for b in range(B)]
    in_maps = []
    for c in range(NCORES):
        b, j = c // 2, c % 2
        m = dict(jmaps[j])
        m["hidT_s"] = hidT[b]
        m["kvT_s"] = kvT[b]
        in_maps.append(m)
    return in_maps


def kernel(hidden_states, key_value_states, q_weight, q_bias,
           kv_weight, kv_bias, out_weight, out_bias, _trace=False):
    if "nc" not in _CACHE:
        _CACHE["nc"] = _build_core_program()
    nc = _CACHE["nc"]
    in_maps = _prep_inputs(hidden_states, key_value_states, q_weight, q_bias,
                           kv_weight, kv_bias, out_weight, out_bias)
    res = bass_utils.run_bass_kernel_spmd(
        nc, in_maps, core_ids=list(range(NCORES)), trace=_trace)
    _CACHE["last_result"] = res
    ob = np.asarray(out_bias, np.float32)
    out = np.empty((B, LQ, D), np.float32)
    for b in range(B):
        p0 = np.asarray(res.results[2 * b]["out_s"], np.float32)
        p1 = np.asarray(res.results[2 * b + 1]["out_s"], np.float32)
        out[b] = p0 + p1 + ob
    return out


# revision 12
# speedup vs baseline: 1.0410x; 1.0410x over previous
"""Trainium2 Bass kernel for nn_BartCrossAttention (B=4, L=1024, D=1024, H=16, HD=64).

v3 sharding: core c -> (batch b = c//2, head-half j = c%2). Each core computes
heads [8j, 8j+8) for ALL 1024 query tokens of its batch, including the K/V/Q
projections restricted to its 512 features, then a PARTIAL out-projection
(contracting only its 512 ctx features). The host sums the two partial outputs
per batch and adds out_bias. No collectives.

v3 changes vs v2 (203.5us):
  - host pre-transposes hid/kv -> hidT/kvT shipped d-major: kills all 128
    PE transposes + identity build, shortens the prologue critical path
  - wk/wq shipped in per-f-block layout [p, f, dd, c] so the f=0 slices can
    land first and K0/Q0 projections start as soon as kvT/hidT stream in
  - DMA issue split across sync/gpsimd/scalar queues, critical tensors first
  - K0/K1/Q0 in the prologue; all other projections spread over the head
    loop as full-array filler (keeps PE HAM at 8/8), one v-site per h0 slot
    (matches the AV consumption deadline exactly), 2 K/Q sites per head after
  - normalization: reciprocal straight off the psum denominator row, then a
    PE broadcast (ones[1,64] x recip row, f32r) + one DVE evict-multiply per
    half; c0 first so the next head's AV frees a ctx tile a slot earlier
  - out partials DMA'd as full [128,1024] tiles round-robin on 3 queues
"""
import sys

for _p in ("/opt/trn_rl_repo",):
    if _p not in sys.path:
        sys.path.insert(0, _p)

import numpy as np
import ml_dtypes

import concourse.bass as bass
import concourse.mybir as mybir
import concourse.tile as tile
from concourse import bacc
import concourse.bass_utils as bass_utils

F32 = mybir.dt.float32
F32R = mybir.dt.float32r
BF16 = mybir.dt.bfloat16
NPBF16 = ml_dtypes.bfloat16

P = 128
D = 1024        # model dim
H = 16          # heads (global)
HPC = 8         # heads per core
FPC = 512       # features per core
NCORES = 8
B, LQ, LK = 4, 1024, 1024

_CACHE = {}


def _build_core_program():
    nc = bacc.Bacc("TRN2", target_bir_lowering=False, debug=False,
                   num_devices=NCORES)

    kvT_s = nc.dram_tensor("kvT_s", [D, LK], BF16, kind="ExternalInput")
    hidT_s = nc.dram_tensor("hidT_s", [D, LQ], BF16, kind="ExternalInput")
    wv_t = nc.dram_tensor("wv_t", [D, FPC], BF16, kind="ExternalInput")
    wkf_t = nc.dram_tensor("wkf_t", [P, 4, 8, P], BF16, kind="ExternalInput")
    wqf_t = nc.dram_tensor("wqf_t", [P, 4, 8, P], BF16, kind="ExternalInput")
    wo_t = nc.dram_tensor("wo_t", [FPC, D], BF16, kind="ExternalInput")
    qkb_d = nc.dram_tensor("qkb", [P, 8], F32, kind="ExternalInput")
    vb_d = nc.dram_tensor("vb", [1, FPC], F32, kind="ExternalInput")
    out_s = nc.dram_tensor("out_s", [LQ, D], BF16, kind="ExternalOutput")

    Exp = mybir.ActivationFunctionType.Exp
    Ident = mybir.ActivationFunctionType.Identity
    add = mybir.AluOpType.add
    mult = mybir.AluOpType.mult

    with tile.TileContext(nc) as tc:
        with (
            tc.tile_pool(name="setup", bufs=1) as setup,
            tc.tile_pool(name="big", bufs=1) as big,
        ):
            # ---- persistent tiles ----
            kvT = big.tile([P, 8, LK], BF16, tag="kvT")    # kv^T [1024,1024]
            hidT = big.tile([P, 8, LQ], BF16, tag="hidT")  # hid^T
            wv = setup.tile([P, 8, FPC], BF16, tag="wv")
            wk = setup.tile([P, 4, 8, P], BF16, tag="wk")  # [p, f, dd, c]
            wq = setup.tile([P, 4, 8, P], BF16, tag="wq")
            wo = setup.tile([P, 4, D], BF16, tag="wo")
            KT = big.tile([P, 4, LK], BF16, tag="KT")      # K^T [512,1024]
            qT = big.tile([P, 4, LQ], BF16, tag="qT")      # Q^T [512,1024]
            v65 = big.tile([P, 8, HPC * 65], BF16, tag="v65")
            ctxT = big.tile([P, 4, LQ], BF16, tag="ctxT")  # ctx^T [512,1024]
            qkb_sb = setup.tile([P, 8], F32, tag="qkb")
            qb_sb = qkb_sb[:, 0:4]
            kb_sb = qkb_sb[:, 4:8]
            vb_row = setup.tile([1, FPC], F32, tag="vb_row")

            # ---- all input DMAs up front, split across 3 issue engines so
            # the critical tensors (wkf0/kvT then wqf0/hidT) stream first ----
            kvT_r = kvT_s.ap().rearrange("(dd p) t -> p dd t", p=P)
            hidT_r = hidT_s.ap().rearrange("(dd p) t -> p dd t", p=P)
            wkf_r = wkf_t.ap()
            wqf_r = wqf_t.ap()

            # per-dd-slab DMAs so the dd-major prologue projections can
            # start accumulating as each slab lands; slabs 0-3 on sync,
            # 4-7 on gpsimd so the two queues stream in parallel
            wv_r = wv_t.ap().rearrange("(dd p) o -> p dd o", p=P)
            nc.sync.dma_start(qkb_sb[:], qkb_d.ap())
            nc.sync.dma_start(wk[:, 0:1, :, :], wkf_r[:, 0:1, :, :])
            for dd in range(4):
                nc.sync.dma_start(kvT[:, dd:dd + 1, :],
                                  kvT_r[:, dd:dd + 1, :])
                nc.sync.dma_start(hidT[:, dd:dd + 1, :],
                                  hidT_r[:, dd:dd + 1, :])
            nc.sync.dma_start(wk[:, 1:2, :, :], wkf_r[:, 1:2, :, :])
            nc.sync.dma_start(wv[:, 0:4, :], wv_r[:, 0:4, :])
            nc.sync.dma_start(wk[:, 2:4, :, :], wkf_r[:, 2:4, :, :])

            nc.gpsimd.dma_start(wq[:, 0:1, :, :], wqf_r[:, 0:1, :, :])
            for dd in range(4, 8):
                nc.gpsimd.dma_start(kvT[:, dd:dd + 1, :],
                                    kvT_r[:, dd:dd + 1, :])
                nc.gpsimd.dma_start(hidT[:, dd:dd + 1, :],
                                    hidT_r[:, dd:dd + 1, :])
            nc.gpsimd.dma_start(wv[:, 4:8, :], wv_r[:, 4:8, :])
            nc.gpsimd.dma_start(vb_row[:], vb_d.ap())
            nc.gpsimd.dma_start(wq[:, 1:4, :, :], wqf_r[:, 1:4, :, :])
            nc.gpsimd.dma_start(wo[:], wo_t.ap().rearrange(
                "(dd p) o -> p dd o", p=P))

            # ---- small setup (gpsimd after its DMA issues; all early) ----
            vbB = setup.tile([P, FPC], F32, tag="vbB")
            nc.gpsimd.partition_broadcast(vbB[:], vb_row[:])

            # ones column (col 64 of each head block) for denominators
            onesF = setup.tile([P, 64], F32, tag="onesF")
            nc.gpsimd.memset(onesF[:], 1.0)
            nc.vector.tensor_copy(
                v65[:].rearrange("p t (h x) -> p t h x", x=65)[:, :, :, 64:65],
                onesF[:].rearrange("p (t h x) -> p t h x", t=8, h=8))

            # ---- projection emitters ----
            def emit_vproj(ti, pool):
                pv = pool.tile([P, FPC], F32, tag="pp", bufs=None,
                               name=f"pv{ti}")
                for dd in range(8):
                    nc.tensor.matmul(
                        pv[:],
                        kvT[:, dd, ti * P:(ti + 1) * P],
                        wv[:, dd, :],
                        start=(dd == 0), stop=(dd == 7),
                    )
                dst = v65[:].rearrange("p t (h x) -> p t h x", x=65)[
                    :, ti, :, 0:64]
                nc.vector.tensor_tensor(dst, pv[:], vbB[:], add)

            def emit_kproj(f, ck, pool):
                pk = pool.tile([P, FPC], F32, tag="pp", bufs=None,
                               name=f"pk{f}_{ck}")
                for dd in range(8):
                    nc.tensor.matmul(
                        pk[:],
                        wk[:, f, dd, :],
                        kvT[:, dd, ck * 512:(ck + 1) * 512],
                        start=(dd == 0), stop=(dd == 7),
                    )
                nc.vector.tensor_scalar(
                    KT[:, f, ck * 512:(ck + 1) * 512], pk[:],
                    kb_sb[:, f:f + 1], None, add)

            def emit_qproj(f, ck, pool):
                pq = pool.tile([P, FPC], F32, tag="pp", bufs=None,
                               name=f"pq{f}_{ck}")
                for dd in range(8):
                    nc.tensor.matmul(
                        pq[:],
                        wq[:, f, dd, :],
                        hidT[:, dd, ck * 512:(ck + 1) * 512],
                        start=(dd == 0), stop=(dd == 7),
                    )
                nc.vector.tensor_scalar(
                    qT[:, f, ck * 512:(ck + 1) * 512], pq[:],
                    qb_sb[:, f:f + 1], None, add)

            # ---- prologue: K f0/f1 + Q f0, dd-major so each matmul only
            # needs one kvT/hidT slab (starts as soon as slabs land) ----
            DD = [0, 4, 1, 5, 2, 6, 3, 7]  # matches 2-queue arrival order
            with tc.tile_pool(name="psmmA", bufs=6, space="PSUM") as psmmA:
                ksites = [(0, 0), (0, 1)]
                qsites = [(0, 0), (0, 1)]
                ktiles = {s: psmmA.tile([P, FPC], F32, tag="pp",
                                        name=f"pk{s[0]}_{s[1]}")
                          for s in ksites}
                qtiles = {s: psmmA.tile([P, FPC], F32, tag="pp",
                                        name=f"pq{s[0]}_{s[1]}")
                          for s in qsites}
                for i, dd in enumerate(DD):
                    for (f, ck) in ksites:
                        nc.tensor.matmul(
                            ktiles[(f, ck)][:],
                            wk[:, f, dd, :],
                            kvT[:, dd, ck * 512:(ck + 1) * 512],
                            start=(i == 0), stop=(i == 7),
                        )
                    for (f, ck) in qsites:
                        nc.tensor.matmul(
                            qtiles[(f, ck)][:],
                            wq[:, f, dd, :],
                            hidT[:, dd, ck * 512:(ck + 1) * 512],
                            start=(i == 0), stop=(i == 7),
                        )
                for (f, ck) in ksites:
                    nc.vector.tensor_scalar(
                        KT[:, f, ck * 512:(ck + 1) * 512],
                        ktiles[(f, ck)][:], kb_sb[:, f:f + 1], None, add)
                for (f, ck) in qsites:
                    nc.vector.tensor_scalar(
                        qT[:, f, ck * 512:(ck + 1) * 512],
                        qtiles[(f, ck)][:], qb_sb[:, f:f + 1], None, add)

            # filler schedule: (h, t) -> emitter. h0 carries one v-site per
            # slot (AV(h0,t) consumes v65[t-1], so site t feeds slot t+1);
            # later heads carry the remaining K/Q f-sites, 2 per head.
            # all 8 v-sites must complete inside h0 (its tail AVs read
            # v65[:,6]/[:,7]); wv lands ~26us so nothing before slot (0,2)
            FILLERS = {
                (0, 0): [("k", 1, 0)],
                (0, 1): [("k", 1, 1)],
                (0, 2): [("v", 0, None)],
                (0, 3): [("v", 1, None), ("v", 2, None)],
                (0, 4): [("v", 3, None)],
                (0, 5): [("v", 4, None), ("v", 5, None)],
                (0, 6): [("v", 6, None)],
                (0, 7): [("v", 7, None)],
                (1, 2): [("q", 1, 0)],
                (1, 5): [("q", 1, 1)],
                (2, 2): [("k", 2, 0)],
                (2, 5): [("k", 2, 1)],
                (3, 2): [("q", 2, 0)],
                (3, 5): [("q", 2, 1)],
                (4, 2): [("k", 3, 0)],
                (4, 5): [("k", 3, 1)],
                (5, 2): [("q", 3, 0)],
                (5, 5): [("q", 3, 1)],
            }

            def emit_filler(h, t, pool):
                for kind, a, b in FILLERS.get((h, t), ()):
                    if kind == "v":
                        emit_vproj(a, pool)
                    elif kind == "k":
                        emit_kproj(a, b, pool)
                    else:
                        emit_qproj(a, b, pool)

            # ---- attention main loop ----
            with (
                tc.tile_pool(name="scp", bufs=2, space="PSUM") as scp,
                tc.tile_pool(name="ctxp", bufs=3, space="PSUM") as ctxp,
                tc.tile_pool(name="psmmB", bufs=1, space="PSUM") as psmmB,
                tc.tile_pool(name="lp", bufs=1) as lp,
            ):
                def emit_norm_recips(h, ctx_pair):
                    # denominator row: psum -> sbuf stage, then fast recip
                    # (custom-DVE recip can't read PSUM directly)
                    rcps = []
                    for c in range(2):
                        stg = lp.tile([1, FPC], F32, tag=f"stg{c}",
                                      name=f"stg{h}_{c}")
                        nc.vector.tensor_copy(stg[:], ctx_pair[c][64:65, :])
                        rcp = lp.tile([1, FPC], F32, tag=f"rcp{c}",
                                      name=f"rcp{h}_{c}")
                        nc.vector.reciprocal_approx_fast(rcp[:], stg[:])
                        rcps.append(rcp)
                    return rcps

                def emit_norm_half(h, ctx_pair, rcps, c):
                    # broadcast the recip row to 64 partitions (gpsimd),
                    # then one DVE evict-multiply into ctxT (bf16)
                    f, rb = h // 2, 64 * (h % 2)
                    bcb = lp.tile([64, FPC], F32, tag=f"bcb{c}",
                                    bufs=2, name=f"bcb{h}_{c}")
                    nc.gpsimd.partition_broadcast(bcb[:], rcps[c][:])
                    nc.vector.tensor_tensor(
                        ctxT[rb:rb + 64, f, c * 512:(c + 1) * 512],
                        ctx_pair[c][0:64, :], bcb[:], mult)

                prev = None  # (h, ctx_pair, rcps) pending normalization
                for h in range(HPC):
                    f, rb = h // 2, 64 * (h % 2)
                    ctx_pair = [ctxp.tile([65, FPC], F32, tag="ctx",
                                          name=f"ctx{h}_{c}")
                                for c in range(2)]
                    ats = []
                    for t in range(8):
                        sc = scp.tile([P, 2 * FPC], F32, tag="sc",
                                      name=f"sc{h}_{t}")
                        for c in range(2):
                            nc.tensor.matmul(
                                sc[:, c * FPC:(c + 1) * FPC],
                                KT[rb:rb + 64, f, t * P:(t + 1) * P],
                                qT[rb:rb + 64, f, c * 512:(c + 1) * 512],
                                start=True, stop=True,
                            )
                        # one 1024-wide exp per t (full ACT column rate)
                        at = lp.tile([P, 2 * FPC], BF16, tag="at",
                                     bufs=4, name=f"at{h}_{t}")
                        nc.scalar.activation(at[:], sc[:], Exp)
                        ats.append(at)
                        # pipelined normalization of the previous head:
                        # c0 at t==0 (frees a ctx tile early), c1 at t==1
                        if prev is not None and t <= 1:
                            ph, pctx, prcps = prev
                            emit_norm_half(ph, pctx, prcps, t)
                            if t == 1:
                                prev = None
                        # full-array projection filler keeps the PE HAM
                        # activity monitor warm (2.4 GHz) during the
                        # half-array attention matmuls
                        emit_filler(h, t, psmmB)
                        # AV lags TWO t-slots: the exp it consumes finished
                        # a full slot ago, so the PE never waits on ACT
                        if t >= 2:
                            for c in range(2):
                                nc.tensor.matmul(
                                    ctx_pair[c][:],
                                    v65[:, t - 2, h * 65:(h + 1) * 65],
                                    ats[t - 2][:, c * FPC:(c + 1) * FPC],
                                    start=(t == 2), stop=False,
                                )
                    for tt in (6, 7):
                        for c in range(2):
                            nc.tensor.matmul(
                                ctx_pair[c][:],
                                v65[:, tt, h * 65:(h + 1) * 65],
                                ats[tt][:, c * FPC:(c + 1) * FPC],
                                start=False, stop=(tt == 7),
                            )
                    rcps = emit_norm_recips(h, ctx_pair)
                    prev = (h, ctx_pair, rcps)
                # final head's normalization (fast path, both halves)
                ph, pctx, prcps = prev
                emit_norm_half(ph, pctx, prcps, 0)
                emit_norm_half(ph, pctx, prcps, 1)

            # ---- epilogue: partial out projection (no bias; host adds) ----
            out_r = out_s.ap().rearrange("(mm p) d -> p mm d", p=P)
            with (
                tc.tile_pool(name="pop", bufs=4, space="PSUM") as pop,
                tc.tile_pool(name="outp", bufs=3) as outp,
            ):
                for m in range(8):
                    ot = outp.tile([P, D], BF16, tag="ot", name=f"ot{m}")
                    for half in range(2):
                        po = pop.tile([P, FPC], F32, tag="po",
                                      name=f"po{m}_{half}")
                        for fj in range(4):
                            nc.tensor.matmul(
                                po[:],
                                ctxT[:, fj, m * P:(m + 1) * P],
                                wo[:, fj, half * 512:(half + 1) * 512],
                                start=(fj == 0), stop=(fj == 3),
                            )
                        if half == 0:
                            nc.scalar.activation(
                                ot[:, half * 512:(half + 1) * 512], po[:],
                                Ident)
                        else:
                            nc.vector.tensor_copy(
                                ot[:, half * 512:(half + 1) * 512], po[:])
                        nc.sync.dma_start(
                            out_r[:, m, half * 512:(half + 1) * 512],
                            ot[:, half * 512:(half + 1) * 512])

    nc.compile()
    return nc


def _prep_inputs(hidden_states, key_value_states, q_weight, q_bias,
                 kv_weight, kv_bias, out_weight, out_bias):
    f32 = np.float32
    hid = np.asarray(hidden_states, f32).reshape(B, LQ, D)
    kv = np.asarray(key_value_states, f32).reshape(B, LK, D)
    scale = f32(1.0 / 8.0)

    # de-interleave kv rows: row e <-> (h=e//128, j=(e%128)//64, d=e%64)
    e = np.arange(2 * D)
    kmask = (e % 128) < 64
    kidx, vidx = e[kmask], e[~kmask]
    kvw = np.asarray(kv_weight, f32)
    kvb = np.asarray(kv_bias, f32)

    wq_full = (np.asarray(q_weight, f32) * scale).T      # [D, D] d x feat
    wk_full = kvw[kidx].T                                # [D, D]
    wv_full = kvw[vidx].T
    wo_full = np.asarray(out_weight, f32).T              # [D, D] feat x out
    qb_full = np.asarray(q_bias, f32) * scale
    kb_full = kvb[kidx]
    vb_full = kvb[vidx]

    def fblock(w):
        # [D, FPC] (d, feat) -> [128 p, 4 f, 8 dd, 128 c] bf16
        return np.ascontiguousarray(
            w.reshape(8, P, 4, P).transpose(1, 2, 0, 3).astype(NPBF16))

    jmaps = []
    for j in range(2):
        s = slice(j * FPC, (j + 1) * FPC)
        jmaps.append({
            "wqf_t": fblock(wq_full[:, s]),
            "wkf_t": fblock(wk_full[:, s]),
            "wv_t": np.ascontiguousarray(wv_full[:, s].astype(NPBF16)),
            "wo_t": np.ascontiguousarray(wo_full[s, :].astype(NPBF16)),
            "qkb": np.ascontiguousarray(np.concatenate(
                [qb_full[s].reshape(4, P).T, kb_full[s].reshape(4, P).T],
                axis=1)),
            "vb": np.ascontiguousarray(vb_full[s].reshape(1, FPC)),
        })
    hidT = [np.ascontiguousarray(hid[b].T.astype(NPBF16)) for b in range(B)]
    kvT = [np.ascontiguousarray(kv[b].T.astype(NPBF16)) for b in range(B)]
    in_maps = []
    for c in range(NCORES):
        b, j = c // 2, c % 2
        m = dict(jmaps[j])
        m["hidT_s"] = hidT[b]
        m["kvT_s"] = kvT[b]
        in_maps.append(m)
    return in_maps


def kernel(hidden_states, key_value_states, q_weight, q_bias,
           kv_weight, kv_bias, out_weight, out_bias, _trace=False):
    if "nc" not in _CACHE:
        _CACHE["nc"] = _build_core_program()
    nc = _CACHE["nc"]
    in_maps = _prep_inputs(hidden_states, key_value_states, q_weight, q_bias,
                           kv_weight, kv_bias, out_weight, out_bias)
    res = bass_utils.run_bass_kernel_spmd(
        nc, in_maps, core_ids=list(range(NCORES)), trace=_trace)
    _CACHE["last_result"] = res
    ob = np.asarray(out_bias, np.float32)
    out = np.empty((B, LQ, D), np.float32)
    for b in range(B):
        p0 = np.asarray(res.results[2 * b]["out_s"], np.float32)
        p1 = np.asarray(res.results[2 * b + 1]["out_s"], np.float32)
        out[b] = p0 + p1 + ob
    return out


# revision 13
# speedup vs baseline: 1.2582x; 1.2087x over previous
"""Trainium2 Bass kernel for nn_BartCrossAttention (B=4, L=1024, D=1024, H=16, HD=64).

v3 sharding: core c -> (batch b = c//2, head-half j = c%2). Each core computes
heads [8j, 8j+8) for ALL 1024 query tokens of its batch, including the K/V/Q
projections restricted to its 512 features, then a PARTIAL out-projection
(contracting only its 512 ctx features). The host sums the two partial outputs
per batch and adds out_bias. No collectives.

v3 changes vs v2 (203.5us):
  - host pre-transposes hid/kv -> hidT/kvT shipped d-major: kills all 128
    PE transposes + identity build, shortens the prologue critical path
  - wk/wq shipped in per-f-block layout [p, f, dd, c] so the f=0 slices can
    land first and K0/Q0 projections start as soon as kvT/hidT stream in
  - DMA issue split across sync/gpsimd/scalar queues, critical tensors first
  - K0/K1/Q0 in the prologue; all other projections spread over the head
    loop as full-array filler (keeps PE HAM at 8/8), one v-site per h0 slot
    (matches the AV consumption deadline exactly), 2 K/Q sites per head after
  - normalization: reciprocal straight off the psum denominator row, then a
    PE broadcast (ones[1,64] x recip row, f32r) + one DVE evict-multiply per
    half; c0 first so the next head's AV frees a ctx tile a slot earlier
  - out partials DMA'd as full [128,1024] tiles round-robin on 3 queues
"""
import sys

for _p in ("/opt/trn_rl_repo",):
    if _p not in sys.path:
        sys.path.insert(0, _p)

import numpy as np
import ml_dtypes

import concourse.bass as bass
import concourse.mybir as mybir
import concourse.tile as tile
from concourse import bacc
import concourse.bass_utils as bass_utils

F32 = mybir.dt.float32
F32R = mybir.dt.float32r
BF16 = mybir.dt.bfloat16
NPBF16 = ml_dtypes.bfloat16

P = 128
D = 1024        # model dim
H = 16          # heads (global)
HPC = 8         # heads per core
FPC = 512       # features per core
NCORES = 8
B, LQ, LK = 4, 1024, 1024

_CACHE = {}


def _build_core_program():
    nc = bacc.Bacc("TRN2", target_bir_lowering=False, debug=False,
                   num_devices=NCORES)

    kvT_s = nc.dram_tensor("kvT_s", [D, LK], BF16, kind="ExternalInput")
    hidT_s = nc.dram_tensor("hidT_s", [D, LQ], BF16, kind="ExternalInput")
    wv_t = nc.dram_tensor("wv_t", [D, FPC], BF16, kind="ExternalInput")
    wkf_t = nc.dram_tensor("wkf_t", [P, 4, 8, P], BF16, kind="ExternalInput")
    wqf_t = nc.dram_tensor("wqf_t", [P, 4, 8, P], BF16, kind="ExternalInput")
    wo_t = nc.dram_tensor("wo_t", [FPC, D], BF16, kind="ExternalInput")
    qkb_d = nc.dram_tensor("qkb", [P, 8], F32, kind="ExternalInput")
    vb_d = nc.dram_tensor("vb", [1, FPC], F32, kind="ExternalInput")
    out_s = nc.dram_tensor("out_s", [LQ, D], BF16, kind="ExternalOutput")

    Exp = mybir.ActivationFunctionType.Exp
    Ident = mybir.ActivationFunctionType.Identity
    add = mybir.AluOpType.add
    mult = mybir.AluOpType.mult

    with tile.TileContext(nc) as tc:
        with (
            tc.tile_pool(name="setup", bufs=1) as setup,
            tc.tile_pool(name="big", bufs=1) as big,
        ):
            # ---- persistent tiles ----
            kvT = big.tile([P, 8, LK], BF16, tag="kvT")    # kv^T [1024,1024]
            hidT = big.tile([P, 8, LQ], BF16, tag="hidT")  # hid^T
            wv = setup.tile([P, 8, FPC], BF16, tag="wv")
            wk = setup.tile([P, 4, 8, P], BF16, tag="wk")  # [p, f, dd, c]
            wq = setup.tile([P, 4, 8, P], BF16, tag="wq")
            wo = setup.tile([P, 4, D], BF16, tag="wo")
            KT = big.tile([P, 4, LK], BF16, tag="KT")      # K^T [512,1024]
            qT = big.tile([P, 4, LQ], BF16, tag="qT")      # Q^T [512,1024]
            v65 = big.tile([P, 8, HPC * 65], BF16, tag="v65")
            ctxT = big.tile([P, 4, LQ], BF16, tag="ctxT")  # ctx^T [512,1024]
            qkb_sb = setup.tile([P, 8], F32, tag="qkb")
            qb_sb = qkb_sb[:, 0:4]
            kb_sb = qkb_sb[:, 4:8]
            vb_row = setup.tile([1, FPC], F32, tag="vb_row")

            # ---- all input DMAs up front, split across 3 issue engines so
            # the critical tensors (wkf0/kvT then wqf0/hidT) stream first ----
            kvT_r = kvT_s.ap().rearrange("(dd p) t -> p dd t", p=P)
            hidT_r = hidT_s.ap().rearrange("(dd p) t -> p dd t", p=P)
            wkf_r = wkf_t.ap()
            wqf_r = wqf_t.ap()

            # per-dd-slab DMAs so the dd-major prologue projections can
            # start accumulating as each slab lands; slabs 0-3 on sync,
            # 4-7 on gpsimd so the two queues stream in parallel
            wv_r = wv_t.ap().rearrange("(dd p) o -> p dd o", p=P)
            nc.sync.dma_start(qkb_sb[:], qkb_d.ap())
            nc.sync.dma_start(wk[:, 0:2, :, :], wkf_r[:, 0:2, :, :])
            for dd in range(4):
                nc.sync.dma_start(kvT[:, dd:dd + 1, :],
                                  kvT_r[:, dd:dd + 1, :])
                nc.sync.dma_start(hidT[:, dd:dd + 1, :],
                                  hidT_r[:, dd:dd + 1, :])
            nc.sync.dma_start(wv[:, 0:4, :], wv_r[:, 0:4, :])
            nc.sync.dma_start(wk[:, 2:4, :, :], wkf_r[:, 2:4, :, :])

            nc.gpsimd.dma_start(wq[:, 0:1, :, :], wqf_r[:, 0:1, :, :])
            for dd in range(4, 8):
                nc.gpsimd.dma_start(kvT[:, dd:dd + 1, :],
                                    kvT_r[:, dd:dd + 1, :])
                nc.gpsimd.dma_start(hidT[:, dd:dd + 1, :],
                                    hidT_r[:, dd:dd + 1, :])
            nc.gpsimd.dma_start(wv[:, 4:8, :], wv_r[:, 4:8, :])
            nc.gpsimd.dma_start(vb_row[:], vb_d.ap())
            nc.gpsimd.dma_start(wq[:, 1:4, :, :], wqf_r[:, 1:4, :, :])
            nc.gpsimd.dma_start(wo[:], wo_t.ap().rearrange(
                "(dd p) o -> p dd o", p=P))

            # ---- small setup (gpsimd after its DMA issues; all early) ----
            vbB = setup.tile([P, FPC], F32, tag="vbB")
            nc.gpsimd.partition_broadcast(vbB[:], vb_row[:])

            # ones column (col 64 of each head block) for denominators
            onesF = setup.tile([P, 64], F32, tag="onesF")
            nc.gpsimd.memset(onesF[:], 1.0)
            nc.vector.tensor_copy(
                v65[:].rearrange("p t (h x) -> p t h x", x=65)[:, :, :, 64:65],
                onesF[:].rearrange("p (t h x) -> p t h x", t=8, h=8))

            # ---- projection emitters ----
            def emit_vproj(ti, pool):
                pv = pool.tile([P, FPC], F32, tag="pp", bufs=None,
                               name=f"pv{ti}")
                for dd in range(8):
                    nc.tensor.matmul(
                        pv[:],
                        kvT[:, dd, ti * P:(ti + 1) * P],
                        wv[:, dd, :],
                        start=(dd == 0), stop=(dd == 7),
                    )
                dst = v65[:].rearrange("p t (h x) -> p t h x", x=65)[
                    :, ti, :, 0:64]
                nc.vector.tensor_tensor(dst, pv[:], vbB[:], add)

            def emit_kproj(f, ck, pool):
                pk = pool.tile([P, FPC], F32, tag="pp", bufs=None,
                               name=f"pk{f}_{ck}")
                for dd in range(8):
                    nc.tensor.matmul(
                        pk[:],
                        wk[:, f, dd, :],
                        kvT[:, dd, ck * 512:(ck + 1) * 512],
                        start=(dd == 0), stop=(dd == 7),
                    )
                nc.vector.tensor_scalar(
                    KT[:, f, ck * 512:(ck + 1) * 512], pk[:],
                    kb_sb[:, f:f + 1], None, add)

            def emit_qproj(f, ck, pool):
                pq = pool.tile([P, FPC], F32, tag="pp", bufs=None,
                               name=f"pq{f}_{ck}")
                for dd in range(8):
                    nc.tensor.matmul(
                        pq[:],
                        wq[:, f, dd, :],
                        hidT[:, dd, ck * 512:(ck + 1) * 512],
                        start=(dd == 0), stop=(dd == 7),
                    )
                nc.vector.tensor_scalar(
                    qT[:, f, ck * 512:(ck + 1) * 512], pq[:],
                    qb_sb[:, f:f + 1], None, add)

            # ---- prologue: K f0/f1 + Q f0, dd-major so each matmul only
            # needs one kvT/hidT slab (starts as soon as slabs land) ----
            DD = [0, 4, 1, 5, 2, 6, 3, 7]  # matches 2-queue arrival order
            with tc.tile_pool(name="psmmA", bufs=6, space="PSUM") as psmmA:
                ksites = [(0, 0), (0, 1), (1, 0), (1, 1)]
                qsites = [(0, 0), (0, 1)]
                ktiles = {s: psmmA.tile([P, FPC], F32, tag="pp",
                                        name=f"pk{s[0]}_{s[1]}")
                          for s in ksites}
                qtiles = {s: psmmA.tile([P, FPC], F32, tag="pp",
                                        name=f"pq{s[0]}_{s[1]}")
                          for s in qsites}
                for i, dd in enumerate(DD):
                    for (f, ck) in ksites:
                        nc.tensor.matmul(
                            ktiles[(f, ck)][:],
                            wk[:, f, dd, :],
                            kvT[:, dd, ck * 512:(ck + 1) * 512],
                            start=(i == 0), stop=(i == 7),
                        )
                    for (f, ck) in qsites:
                        nc.tensor.matmul(
                            qtiles[(f, ck)][:],
                            wq[:, f, dd, :],
                            hidT[:, dd, ck * 512:(ck + 1) * 512],
                            start=(i == 0), stop=(i == 7),
                        )
                for (f, ck) in ksites:
                    nc.vector.tensor_scalar(
                        KT[:, f, ck * 512:(ck + 1) * 512],
                        ktiles[(f, ck)][:], kb_sb[:, f:f + 1], None, add)
                for (f, ck) in qsites:
                    nc.vector.tensor_scalar(
                        qT[:, f, ck * 512:(ck + 1) * 512],
                        qtiles[(f, ck)][:], qb_sb[:, f:f + 1], None, add)

            # filler schedule: (h, t) -> emitter. h0 carries one v-site per
            # slot (AV(h0,t) consumes v65[t-1], so site t feeds slot t+1);
            # later heads carry the remaining K/Q f-sites, 2 per head.
            # all 8 v-sites must complete inside h0 (its tail AVs read
            # v65[:,6]/[:,7]); wv lands ~26us so nothing before slot (0,2)
            FILLERS = {
                (0, 2): [("v", 0, None)],
                (0, 3): [("v", 1, None), ("v", 2, None)],
                (0, 4): [("v", 3, None)],
                (0, 5): [("v", 4, None), ("v", 5, None)],
                (0, 6): [("v", 6, None)],
                (0, 7): [("v", 7, None)],
                (1, 2): [("q", 1, 0)],
                (1, 5): [("q", 1, 1)],
                (2, 2): [("k", 2, 0)],
                (2, 5): [("k", 2, 1)],
                (3, 2): [("q", 2, 0)],
                (3, 5): [("q", 2, 1)],
                (4, 2): [("k", 3, 0)],
                (4, 5): [("k", 3, 1)],
                (5, 2): [("q", 3, 0)],
                (5, 5): [("q", 3, 1)],
            }

            def emit_filler(h, t, pool):
                for kind, a, b in FILLERS.get((h, t), ()):
                    if kind == "v":
                        emit_vproj(a, pool)
                    elif kind == "k":
                        emit_kproj(a, b, pool)
                    else:
                        emit_qproj(a, b, pool)

            # ---- attention main loop ----
            with (
                tc.tile_pool(name="scp", bufs=2, space="PSUM") as scp,
                tc.tile_pool(name="ctxp", bufs=3, space="PSUM") as ctxp,
                tc.tile_pool(name="psmmB", bufs=1, space="PSUM") as psmmB,
                tc.tile_pool(name="lp", bufs=1) as lp,
            ):
                def emit_norm_recips(h, ctx_pair):
                    # denominator row: psum -> sbuf stage, then fast recip
                    # (custom-DVE recip can't read PSUM directly)
                    rcps = []
                    for c in range(2):
                        stg = lp.tile([1, FPC], F32, tag=f"stg{c}",
                                      name=f"stg{h}_{c}")
                        nc.vector.tensor_copy(stg[:], ctx_pair[c][64:65, :])
                        rcp = lp.tile([1, FPC], F32, tag=f"rcp{c}",
                                      name=f"rcp{h}_{c}")
                        nc.vector.reciprocal_approx_fast(rcp[:], stg[:])
                        rcps.append(rcp)
                    return rcps

                def emit_norm_half(h, ctx_pair, rcps, c):
                    # broadcast the recip row to 64 partitions (gpsimd),
                    # then one DVE evict-multiply into ctxT (bf16)
                    f, rb = h // 2, 64 * (h % 2)
                    bcb = lp.tile([64, FPC], F32, tag=f"bcb{c}",
                                    bufs=2, name=f"bcb{h}_{c}")
                    nc.gpsimd.partition_broadcast(bcb[:], rcps[c][:])
                    nc.vector.tensor_tensor(
                        ctxT[rb:rb + 64, f, c * 512:(c + 1) * 512],
                        ctx_pair[c][0:64, :], bcb[:], mult)

                prev = None  # (h, ctx_pair, rcps) pending normalization
                for h in range(HPC):
                    f, rb = h // 2, 64 * (h % 2)
                    ctx_pair = [ctxp.tile([65, FPC], F32, tag="ctx",
                                          name=f"ctx{h}_{c}")
                                for c in range(2)]
                    ats = []
                    for t in range(8):
                        sc = scp.tile([P, 2 * FPC], F32, tag="sc",
                                      name=f"sc{h}_{t}")
                        for c in range(2):
                            nc.tensor.matmul(
                                sc[:, c * FPC:(c + 1) * FPC],
                                KT[rb:rb + 64, f, t * P:(t + 1) * P],
                                qT[rb:rb + 64, f, c * 512:(c + 1) * 512],
                                start=True, stop=True,
                            )
                        # one 1024-wide exp per t (full ACT column rate)
                        at = lp.tile([P, 2 * FPC], BF16, tag="at",
                                     bufs=4, name=f"at{h}_{t}")
                        nc.scalar.activation(at[:], sc[:], Exp)
                        ats.append(at)
                        # pipelined normalization of the previous head:
                        # c0 at t==0 (frees a ctx tile early), c1 at t==1
                        if prev is not None and t <= 1:
                            ph, pctx, prcps = prev
                            emit_norm_half(ph, pctx, prcps, t)
                            if t == 1:
                                prev = None
                        # full-array projection filler keeps the PE HAM
                        # activity monitor warm (2.4 GHz) during the
                        # half-array attention matmuls
                        emit_filler(h, t, psmmB)
                        # AV lags TWO t-slots: the exp it consumes finished
                        # a full slot ago, so the PE never waits on ACT
                        if t >= 2:
                            for c in range(2):
                                nc.tensor.matmul(
                                    ctx_pair[c][:],
                                    v65[:, t - 2, h * 65:(h + 1) * 65],
                                    ats[t - 2][:, c * FPC:(c + 1) * FPC],
                                    start=(t == 2), stop=False,
                                )
                    for tt in (6, 7):
                        for c in range(2):
                            nc.tensor.matmul(
                                ctx_pair[c][:],
                                v65[:, tt, h * 65:(h + 1) * 65],
                                ats[tt][:, c * FPC:(c + 1) * FPC],
                                start=False, stop=(tt == 7),
                            )
                    rcps = emit_norm_recips(h, ctx_pair)
                    prev = (h, ctx_pair, rcps)
                # final head's normalization (fast path, both halves)
                ph, pctx, prcps = prev
                emit_norm_half(ph, pctx, prcps, 0)
                emit_norm_half(ph, pctx, prcps, 1)

            # ---- epilogue: partial out projection (no bias; host adds) ----
            out_r = out_s.ap().rearrange("(mm p) d -> p mm d", p=P)
            with (
                tc.tile_pool(name="pop", bufs=4, space="PSUM") as pop,
                tc.tile_pool(name="outp", bufs=3) as outp,
            ):
                for m in range(8):
                    ot = outp.tile([P, D], BF16, tag="ot", name=f"ot{m}")
                    for half in range(2):
                        po = pop.tile([P, FPC], F32, tag="po",
                                      name=f"po{m}_{half}")
                        for fj in range(4):
                            nc.tensor.matmul(
                                po[:],
                                ctxT[:, fj, m * P:(m + 1) * P],
                                wo[:, fj, half * 512:(half + 1) * 512],
                                start=(fj == 0), stop=(fj == 3),
                            )
                        if half == 0:
                            nc.scalar.activation(
                                ot[:, half * 512:(half + 1) * 512], po[:],
                                Ident)
                        else:
                            nc.vector.tensor_copy(
                                ot[:, half * 512:(half + 1) * 512], po[:])
                        nc.sync.dma_start(
                            out_r[:, m, half * 512:(half + 1) * 512],
                            ot[:, half * 512:(half + 1) * 512])

    nc.compile()
    return nc


def _prep_inputs(hidden_states, key_value_states, q_weight, q_bias,
                 kv_weight, kv_bias, out_weight, out_bias):
    f32 = np.float32
    hid = np.asarray(hidden_states, f32).reshape(B, LQ, D)
    kv = np.asarray(key_value_states, f32).reshape(B, LK, D)
    scale = f32(1.0 / 8.0)

    # de-interleave kv rows: row e <-> (h=e//128, j=(e%128)//64, d=e%64)
    e = np.arange(2 * D)
    kmask = (e % 128) < 64
    kidx, vidx = e[kmask], e[~kmask]
    kvw = np.asarray(kv_weight, f32)
    kvb = np.asarray(kv_bias, f32)

    wq_full = (np.asarray(q_weight, f32) * scale).T      # [D, D] d x feat
    wk_full = kvw[kidx].T                                # [D, D]
    wv_full = kvw[vidx].T
    wo_full = np.asarray(out_weight, f32).T              # [D, D] feat x out
    qb_full = np.asarray(q_bias, f32) * scale
    kb_full = kvb[kidx]
    vb_full = kvb[vidx]

    def fblock(w):
        # [D, FPC] (d, feat) -> [128 p, 4 f, 8 dd, 128 c] bf16
        return np.ascontiguousarray(
            w.reshape(8, P, 4, P).transpose(1, 2, 0, 3).astype(NPBF16))

    jmaps = []
    for j in range(2):
        s = slice(j * FPC, (j + 1) * FPC)
        jmaps.append({
            "wqf_t": fblock(wq_full[:, s]),
            "wkf_t": fblock(wk_full[:, s]),
            "wv_t": np.ascontiguousarray(wv_full[:, s].astype(NPBF16)),
            "wo_t": np.ascontiguousarray(wo_full[s, :].astype(NPBF16)),
            "qkb": np.ascontiguousarray(np.concatenate(
                [qb_full[s].reshape(4, P).T, kb_full[s].reshape(4, P).T],
                axis=1)),
            "vb": np.ascontiguousarray(vb_full[s].reshape(1, FPC)),
        })
    hidT = [np.ascontiguousarray(hid[b].T.astype(NPBF16)) for b in range(B)]
    kvT = [np.ascontiguousarray(kv[b].T.astype(NPBF16)) for b in range(B)]
    in_maps = []
    for c in range(NCORES):
        b, j = c // 2, c % 2
        m = dict(jmaps[j])
        m["hidT_s"] = hidT[b]
        m["kvT_s"] = kvT[b]
        in_maps.append(m)
    return in_maps


def kernel(hidden_states, key_value_states, q_weight, q_bias,
           kv_weight, kv_bias, out_weight, out_bias, _trace=False):
    if "nc" not in _CACHE:
        _CACHE["nc"] = _build_core_program()
    nc = _CACHE["nc"]
    in_maps = _prep_inputs(hidden_states, key_value_states, q_weight, q_bias,
                           kv_weight, kv_bias, out_weight, out_bias)
    res = bass_utils.run_bass_kernel_spmd(
        nc, in_maps, core_ids=list(range(NCORES)), trace=_trace)
    _CACHE["last_result"] = res
    ob = np.asarray(out_bias, np.float32)
    out = np.empty((B, LQ, D), np.float32)
    for b in range(B):
        p0 = np.asarray(res.results[2 * b]["out_s"], np.float32)
        p1 = np.asarray(res.results[2 * b + 1]["out_s"], np.float32)
        out[b] = p0 + p1 + ob
    return out
